# revision 1
# baseline (speedup 1.0000x reference)
"""Trainium2 Bass kernel for AttentionFusion (B=4, T=4, H=W=32, C=128).

Sharding: 8 cores = batch (4) x query-half (2). Each core computes full
attention for 2048 query rows of one batch element against all 4096 keys
of that element. No cross-core communication.

Per-core pipeline (raw Bass, manual semaphores; this walrus build allows
only one sync-wait per instruction, so waits are standalone wait_ge):
  PE : K/Q/V projections, S^T = K_tile^T @ Q_block (fp32r), rowsum via
       ones-matmul, O += V_tile @ P_tile (bf16), Wo projection, recip
       broadcast.
  ACT: exp(scale*S) PSUM->SBUF (bf16), one op per two 512-col PSUM banks.
  DVE: PSUM->SBUF copies (+bias adds), reciprocal, final normalize.
  POOL: DMA.
"""
import sys

sys.path.insert(0, "/opt/trn_rl_repo")

import numpy as np
import ml_dtypes

import concourse.bass as bass
import concourse.mybir as mybir
from concourse.bass_utils import run_bass_kernel_spmd

f32 = mybir.dt.float32
f32r = mybir.dt.float32r
bf16 = mybir.dt.bfloat16
f16 = mybir.dt.float16

B, T, C, H, W = 4, 4, 128, 32, 32
N = T * H * W            # 4096 keys per batch element
NLOC = N // 2            # 2048 query rows per core
NB = NLOC // 512         # 4 column blocks of 512 queries
MT = N // 128            # 32 key tiles
NG = MT // 2             # 16 exp groups per block (2 tiles each)
SCALE = float(C) ** -0.5
NJUNK = 28               # HAM warm-up matmuls during the input-DMA wait

N_CORES = 8


def _build(stage="full"):
    nc = bass.Bass("TRN2")

    xs = nc.declare_dram_parameter("xs", [C, NLOC], f16, isOutput=False)
    xt = nc.declare_dram_parameter("xt", [C, N], f16, isOutput=False)
    w3 = nc.declare_dram_parameter("w3", [C, 3 * C], f16, isOutput=False)  # wqT|wkT|woT
    b3 = nc.declare_dram_parameter("b3", [C, 3], f32, isOutput=False)       # bq|bk|bo_eff
    onesr = nc.declare_dram_parameter("onesr", [1, C], f16, isOutput=False)
    wvob = nc.declare_dram_parameter("wvob", [C, C + 1], f16, isOutput=False)  # ones_col|wvT
    out = nc.declare_dram_parameter("out", [C, NLOC], f32, isOutput=True)

    N_IN_DMAS = 6

    # ---- precomputed semaphore schedules (must mirror emission order) ----
    pe = NJUNK
    k_mm, vt_mm, q_mm = {}, {}, {}
    for j in range(8):
        pe += 1; k_mm[j] = pe
    for mt in range(MT):
        pe += 1; vt_mm[mt] = pe
    for j in range(4):
        pe += 1; q_mm[j] = pe
    st_cnt, pv_cnt, rsP_cnt, rb_cnt, y_cnt = {}, {}, {}, {}, {}
    for nb in range(NB):
        for g in range(NG):
            if g == 4 and nb >= 1:
                pe += 1; rb_cnt[nb - 1] = pe
                pe += 1; y_cnt[nb - 1] = pe
            pe += 1; st_cnt[(nb, 2 * g)] = pe
            pe += 1; st_cnt[(nb, 2 * g + 1)] = pe
            if g >= 1:
                for k in (2 * g - 2, 2 * g - 1):
                    pe += 1; pv_cnt[(nb, k)] = pe
            if g >= 6 and g % 2 == 0:
                pe += 1; rsP_cnt[(nb, (g - 6) // 2)] = pe
        for k in (MT - 2, MT - 1):
            pe += 1; pv_cnt[(nb, k)] = pe
        pe += 1; rsP_cnt[(nb, 5)] = pe
        pe += 1; rsP_cnt[(nb, 6)] = pe
        pe += 1; rsP_cnt[(nb, 7)] = pe
        if nb == NB - 1:
            pe += 1; rb_cnt[nb] = pe
            pe += 1; y_cnt[nb] = pe

    act_c = 0
    vtcopy = {}
    for mt in range(MT):
        act_c += 1; vtcopy[mt] = act_c
    dve = 0
    kcopy, qcopy = {}, {}
    for j in range(8):
        dve += 1; kcopy[j] = dve
    for j in range(4):
        dve += 1; qcopy[j] = dve
    PROJ_DVE = dve  # 12
    rs_free, rcr_ready, o_ready, rb_ready, y_ready = {}, {}, {}, {}, {}
    padd, qadd = {}, {}
    for nb in range(NB):
        for p in range(NG):
            dve += 1; padd[(nb, p)] = dve
            if p % 2 == 1:
                dve += 1; qadd[(nb, p // 2)] = dve
        dve += 1; o_ready[nb] = dve    # O copy (frees o_ps early)
        dve += 1; rs_free[nb] = dve    # rs_ps -> sbuf copy (frees rs_ps fast)
        dve += 1                        # reciprocal
        dve += 1; rcr_ready[nb] = dve  # f32->f16 cast of recip
        dve += 1; rb_ready[nb] = dve   # rb copy
        dve += 1                       # mul
        dve += 1; y_ready[nb] = dve    # bias add -> Y block done

    from contextlib import ExitStack
    ctx = ExitStack()
    with ctx:
        def sb(name, shape, dt):
            return ctx.enter_context(nc.sbuf_tensor(name, shape, dt))
        def ps(name, shape, dt):
            return ctx.enter_context(nc.psum_tensor(name, shape, dt))
        s_xs = sb("s_xs", [C, NLOC], f16)
        s_xt = sb("s_xt", [C, N], f16)
        s_w3 = sb("s_w3", [C, 3 * C], f16)
        s_b3 = sb("s_b3", [C, 3], f32)
        s_onesr = sb("s_onesr", [1, C], f16)
        s_wvob = sb("s_wvob", [C, C + 1], f16)
        s_K = sb("s_K", [C, N], f16)
        s_Q = sb("s_Q", [C, NLOC], f16)
        s_VT = sb("s_VT", [C, N], f16)          # 32 tiles of [128,128]
        s_PT = sb("s_PT", [C, 32 * 512], f16)    # 8 ring slots of [128,512]
        s_PS = sb("s_PS", [C, 8 * 512], f16)    # 8 pair-sum ring slots
        s_QS = sb("s_QS", [C, 4 * 512], f16)    # 4 quad-sum ring slots
        s_O = sb("s_O", [C, 512], f16)
        s_rs32 = sb("s_rs32", [1, 512], f32)
        s_rc = sb("s_rc", [1, 512], f32)
        s_rc16 = sb("s_rc16", [1, 512], f16)
        s_rb = sb("s_rb", [C, 512], f16)
        s_ytmp = sb("s_ytmp", [C, 512], f32)
        s_Y = sb("s_Y", [C, NLOC], f32)
        s_warm = sb("s_warm", [C, 512], f16)   # never written; HAM warm-up fuel
        st_ps0 = ps("st_ps0", [C, 1024], f32)
        st_ps1 = ps("st_ps1", [C, 1024], f32)
        o_ps = ps("o_ps", [C, 512], f32)
        rs_ps = ps("rs_ps", [1, 512], f32)
        rb_ps = ps("rb_ps", [C, 512], f32)
        y_ps = ps("y_ps", [C, 512], f32)
        dma_sem = ctx.enter_context(nc.semaphore("dma_sem"))
        pe_sem = ctx.enter_context(nc.semaphore("pe_sem"))
        act_sem = ctx.enter_context(nc.semaphore("act_sem"))
        dve_sem = ctx.enter_context(nc.semaphore("dve_sem"))
        block = ctx.enter_context(nc.Block())

        st_ps = [st_ps0, st_ps1]
        vt_slots = [rb_ps, y_ps]  # VT projection scratch: 8 slots of [128,128]

        def st_slot(j):
            # 4 rotating [128,512] psum slots used by K/Q proj and main ST
            return st_ps[(j // 2) % 2][:, (j % 2) * 512:(j % 2) * 512 + 512]

        def pt_slot(kglob, ntiles=1):
            s = kglob % 32
            return s_PT[:, s * 512:(s + ntiles) * 512]

        @block.tensor
        def _(tensor):
            # keep the PE busy during the DMA wait so the HAM clock gate
            # stays at 8/8; results are garbage and cleared by start=True
            for i in range(NJUNK):
                nc.tensor.matmul(st_slot(i), s_warm[:, 0:C],
                                 s_warm[:], start=True,
                                 stop=True).then_inc(pe_sem, 1)
            tensor.wait_ge(dma_sem, 16 * N_IN_DMAS)
            tensor.wait_ge(pe_sem, NJUNK)
            # K projection: K[c, m] = wkT.T @ xt
            for j in range(8):
                if j >= 4:
                    tensor.wait_ge(dve_sem, kcopy[j - 4])
                nc.tensor.matmul(st_slot(j), s_w3[:, C:2 * C], s_xt[:, j * 512:(j + 1) * 512],
                                 start=True, stop=True).then_inc(pe_sem, 1)
            # V^T tiles: VT[m, c] = xt_tile.T @ wvT   (bf16)
            for mt in range(MT):
                if mt >= 2:
                    tensor.wait_ge(act_sem, vtcopy[mt - 2])
                slot = vt_slots[mt % 2][:, ((mt // 2) % 2) * 128:((mt // 2) % 2) * 128 + 128]
                nc.tensor.matmul(slot, s_xt[:, mt * 128:(mt + 1) * 128], s_wvob[:, 1:],
                                 start=True, stop=True).then_inc(pe_sem, 1)
            # Q projection
            for j in range(4):
                tensor.wait_ge(dve_sem, kcopy[4 + j])
                nc.tensor.matmul(st_slot(8 + j), s_w3[:, 0:C], s_xs[:, j * 512:(j + 1) * 512],
                                 start=True, stop=True).then_inc(pe_sem, 1)

            def pv(nb, k):
                pt = pt_slot(nb * MT + k)
                nc.tensor.matmul(o_ps[:], s_VT[:, k * 128:(k + 1) * 128], pt,
                                 start=(k == 0), stop=(k == MT - 1)).then_inc(pe_sem, 1)

            def rsP(nb, q):
                tensor.wait_ge(dve_sem, qadd[(nb, q)])
                if q == 0 and nb >= 1:
                    tensor.wait_ge(dve_sem, rs_free[nb - 1])  # rs_ps free
                nc.tensor.matmul(rs_ps[:], s_wvob[:, 0:1],
                                 s_QS[:, (q % 4) * 512:(q % 4) * 512 + 512],
                                 start=(q == 0), stop=(q == 7)).then_inc(pe_sem, 1)

            def rb_y(b):
                tensor.wait_ge(dve_sem, rcr_ready[b])
                nc.tensor.matmul(rb_ps[:], s_onesr[:], s_rc16[:],
                                 start=True, stop=True).then_inc(pe_sem, 1)
                tensor.wait_ge(dve_sem, o_ready[b])
                nc.tensor.matmul(y_ps[:], s_w3[:, 2 * C:3 * C], s_O[:],
                                 start=True, stop=True).then_inc(pe_sem, 1)

            for nb in range(NB):
                q_rhs = s_Q[:, nb * 512:(nb + 1) * 512]
                for g in range(NG):
                    if g == 0:
                        if nb == 0:
                            tensor.wait_ge(dve_sem, PROJ_DVE)  # proj copies done
                        else:
                            tensor.wait_ge(act_sem, MT + nb * NG)   # prev block exps done
                    if g == 4 and nb >= 1:
                        rb_y(nb - 1)
                    # pt ring: pair adds of pair g-4 must be done before the
                    # exp that this ST pair enables overwrites those slots
                    if nb >= 1:
                        tensor.wait_ge(dve_sem, padd[(nb - 1, g)])
                    for mt in (2 * g, 2 * g + 1):
                        nc.tensor.matmul(st_slot(mt), s_K[:, mt * 128:(mt + 1) * 128],
                                         q_rhs, start=True, stop=True).then_inc(pe_sem, 1)
                    if g >= 1:
                        tensor.wait_ge(act_sem, MT + nb * NG + g)   # exp group g-1 done
                        if g == 1 and nb >= 1:
                            tensor.wait_ge(dve_sem, o_ready[nb - 1])  # o_ps free
                        pv(nb, 2 * g - 2)
                        pv(nb, 2 * g - 1)
                    if g >= 6 and g % 2 == 0:
                        rsP(nb, (g - 6) // 2)
                tensor.wait_ge(act_sem, MT + nb * NG + NG)
                pv(nb, MT - 2)
                pv(nb, MT - 1)
                rsP(nb, 5)
                rsP(nb, 6)
                rsP(nb, 7)
            rb_y(NB - 1)

        @block.scalar
        def _(scalar):
            for mt in range(MT):
                scalar.wait_ge(pe_sem, vt_mm[mt])
                slot = vt_slots[mt % 2][:, ((mt // 2) % 2) * 128:((mt // 2) % 2) * 128 + 128]
                nc.scalar.copy(s_VT[:, mt * 128:(mt + 1) * 128],
                               slot).then_inc(act_sem, 1)
            for nb in range(NB):
                for g in range(NG):
                    kglob = nb * MT + 2 * g
                    scalar.wait_ge(pe_sem, st_cnt[(nb, 2 * g + 1)])
                    if kglob >= 32:
                        # PT ring slots freed once pv of tile kglob-7 issued
                        prev = kglob - 31
                        scalar.wait_ge(pe_sem, pv_cnt[(prev // MT, prev % MT)])
                    nc.scalar.activation(pt_slot(kglob, 2), st_ps[g % 2][:],
                                         mybir.ActivationFunctionType.Exp,
                                         scale=SCALE).then_inc(act_sem, 1)

        @block.vector
        def _(vector):
            # projection copies
            for j in range(8):
                vector.wait_ge(pe_sem, k_mm[j])
                vector.tensor_scalar_add(s_K[:, j * 512:(j + 1) * 512], st_slot(j),
                                         s_b3[:, 1:2]).then_inc(dve_sem, 1)
            for j in range(4):
                vector.wait_ge(pe_sem, q_mm[j])
                vector.tensor_scalar_add(s_Q[:, j * 512:(j + 1) * 512], st_slot(8 + j),
                                         s_b3[:, 0:1]).then_inc(dve_sem, 1)
            # pair adds + per-block epilogue
            for nb in range(NB):
                for p in range(NG):
                    vector.wait_ge(pe_sem, pv_cnt[(nb, 2 * p + 1)])
                    if p >= 8:
                        vector.wait_ge(pe_sem, rsP_cnt[(nb, (p - 8) // 2)])
                    elif nb >= 1:
                        vector.wait_ge(pe_sem, rsP_cnt[(nb - 1, (NG - 8 + p) // 2)])
                    kg = nb * MT + 2 * p
                    nc.vector.tensor_add(
                        s_PS[:, (p % 8) * 512:(p % 8) * 512 + 512],
                        pt_slot(kg), pt_slot(kg + 1)).then_inc(dve_sem, 1)
                    if p % 2 == 1:
                        q = p // 2
                        if q >= 4:
                            vector.wait_ge(pe_sem, rsP_cnt[(nb, q - 4)])
                        elif nb >= 1:
                            vector.wait_ge(pe_sem, rsP_cnt[(nb - 1, q + 4)])
                        vector.wait_ge(dve_sem, padd[(nb, p)])
                        nc.vector.tensor_add(
                            s_QS[:, (q % 4) * 512:(q % 4) * 512 + 512],
                            s_PS[:, ((2 * q) % 8) * 512:((2 * q) % 8) * 512 + 512],
                            s_PS[:, ((2 * q + 1) % 8) * 512:((2 * q + 1) % 8) * 512 + 512],
                        ).then_inc(dve_sem, 1)
                vector.wait_ge(pe_sem, pv_cnt[(nb, MT - 1)])
                vector.tensor_copy(s_O[:], o_ps[:]).then_inc(dve_sem, 1)
                vector.wait_ge(pe_sem, rsP_cnt[(nb, 7)])
                vector.tensor_copy(s_rs32[:], rs_ps[:]).then_inc(dve_sem, 1)
                vector.wait_ge(dve_sem, rs_free[nb])
                vector.reciprocal(s_rc[:], s_rs32[:]).then_inc(dve_sem, 1)
                vector.wait_ge(dve_sem, rs_free[nb] + 1)
                vector.tensor_copy(s_rc16[:], s_rc[:]).then_inc(dve_sem, 1)
                vector.wait_ge(pe_sem, rb_cnt[nb])
                vector.tensor_copy(s_rb[:], rb_ps[:]).then_inc(dve_sem, 1)
                vector.wait_ge(pe_sem, y_cnt[nb])
                vector.wait_ge(dve_sem, rb_ready[nb])
                vector.tensor_mul(s_ytmp[:], y_ps[:], s_rb[:]).then_inc(dve_sem, 1)
                vector.wait_ge(dve_sem, rb_ready[nb] + 1)
                vector.tensor_scalar_add(s_Y[:, nb * 512:(nb + 1) * 512], s_ytmp[:],
                                         s_b3[:, 2:3]).then_inc(dve_sem, 1)

        @block.gpsimd
        def _(gpsimd):
            gpsimd.dma_start(s_xt[:], xt[:]).then_inc(dma_sem, 16)
            gpsimd.dma_start(s_xs[:], xs[:]).then_inc(dma_sem, 16)
            gpsimd.dma_start(s_w3[:], w3[:]).then_inc(dma_sem, 16)
            gpsimd.dma_start(s_b3[:], b3[:]).then_inc(dma_sem, 16)
            gpsimd.dma_start(s_onesr[:], onesr[:]).then_inc(dma_sem, 16)
            gpsimd.dma_start(s_wvob[:], wvob[:]).then_inc(dma_sem, 16)
            for nb in range(NB):
                gpsimd.wait_ge(dve_sem, y_ready[nb])
                gpsimd.dma_start(out[:, nb * 512:(nb + 1) * 512],
                                 s_Y[:, nb * 512:(nb + 1) * 512]).then_inc(dma_sem, 16)

    return nc


def _make_in_maps(spatial_features, temporal_features, Wq, bq, Wk, bk, Wv, bv, Wo, bo):
    f = np.float32
    bf = np.float16
    w3 = np.ascontiguousarray(np.concatenate([Wq.T, Wk.T, Wo.T], axis=1)).astype(bf)
    bo_eff = (Wo @ bv + bo).astype(f)
    b3 = np.ascontiguousarray(np.stack([bq, bk, bo_eff], axis=1)).astype(f)  # [C, 3]
    onesr = np.ones((1, C), bf)
    wvob = np.ascontiguousarray(
        np.concatenate([np.ones((C, 1), f), Wv.T], axis=1)).astype(bf)

    in_maps = []
    for core in range(N_CORES):
        b, half = core // 2, core % 2
        xs_ = np.ascontiguousarray(
            spatial_features[b, 2 * half:2 * half + 2]      # [2, C, H, W]
            .transpose(1, 0, 2, 3).reshape(C, NLOC)).astype(bf)
        xt_ = np.ascontiguousarray(temporal_features[b].reshape(C, N)).astype(bf)
        in_maps.append({
            "xs": xs_,
            "xt": xt_,
            "w3": w3,
            "b3": b3,
            "onesr": onesr,
            "wvob": wvob,
        })
    return in_maps


_CACHED = {}


def _run(in_maps, trace=False):
    import os
    stage = os.environ.get("KSTAGE", "full")
    if _CACHED.get("stage") != stage:
        _CACHED["nc"] = _build(stage)
        _CACHED["stage"] = stage
    return run_bass_kernel_spmd(_CACHED["nc"], in_maps, list(range(N_CORES)), trace=trace)


def kernel(spatial_features, temporal_features, Wq, bq, Wk, bk, Wv, bv, Wo, bo):
    args = [np.asarray(a) for a in (spatial_features, temporal_features,
                                    Wq, bq, Wk, bk, Wv, bv, Wo, bo)]
    in_maps = _make_in_maps(*args)
    res = _run(in_maps)
    out = np.empty((B, C, T, H, W), np.float32)
    for core in range(N_CORES):
        b, half = core // 2, core % 2
        y = res.results[core]["out"]                        # [C, NLOC]
        out[b, :, 2 * half:2 * half + 2] = np.asarray(y).reshape(C, 2, H, W)
    return out

